# revision 1
# baseline (speedup 1.0000x reference)
"""Additive attention (Bahdanau-style) on 8 TRN2 NeuronCores.

Reference computation (S=1024, B=64, QK=H=DV=1024):
    q = queries @ W_q.T          [S,B,H]
    k = keys    @ W_k.T          [S,B,H]
    f = tanh(q + k)              [S,B,H]
    scores = f @ w_v.T           [S,B,1]
    attn = softmax(scores, axis=S)
    out[b,d] = sum_s attn[s,b] * values[s,0,d]    -> [B,DV]

Strategy: pure data parallel over B (8 batches per core), weights/values
replicated, no collectives.  Per core the dominant work is one fused
matmul [S*BL, 2*QK] @ [2*QK, H] (queries/keys concatenated along the
contraction dim), computed in bf16 with the contraction dim
pre-transposed onto SBUF partitions host-side.  z is produced in
[h, sb] layout (weights stationary); tanh runs on the scalar engine out
of PSUM; scores are produced directly transposed ([sb,1] columns) by
f-stationary matmuls against the w_v column, so exp'd scores land in
the [s, b] layout the final attn^T @ V matmul needs with no shuffle.
Softmax normalization folds into the output copy as a per-partition
1/den scale.

This walrus build rejects engine instructions with more than one
embedded sync wait, so the program must be built as bacc.Bacc and run
through Bacc.compile(): its generate_event_semaphores pass splits
excess on_wait entries onto standalone InstEventSemaphore carriers.
(The optional act_observe machinery below predates that discovery and
is off by default.)

Measured on TRN2: ~488-492 us HW exec (8 cores SPMD), vs a ~464 us
per-core PE-busy floor (446 us z-matmul stream at the bf16 1-cycle/row
rate + ~18 us score/output matmuls) plus ~27 us fixed preamble, DMA
pipeline fill, and drain.  The PE stream runs with <2 us of total gaps;
prologue experiments (earlier PE start, parallel trigger dispatch,
finer first-chunk DMAs) all converged to the same total because the
ramp is DMA-pipeline-fill bound.  Note the chip clock is bimodal under
sustained benchmarking: ~2.4 GHz rested vs ~2.0 GHz hot (+20% wall).
"""

import numpy as np
import ml_dtypes

P = 128
CORES = 8
CHUNK_W = 512   # token-chunk width; build_nc and prep_in_maps must agree

FULL_S, FULL_B, FULL_QK, FULL_H, FULL_DV = 1024, 64, 1024, 1024, 1024


def build_nc(S=FULL_S, BL=FULL_B // CORES, QK2=2 * FULL_QK, H=FULL_H,
             DV=FULL_DV, CW=CHUNK_W, XT_BUFS=4, Z_BUFS=3, use_observers=False):
    """Build the single-core Bacc program (same NEFF runs SPMD on all cores)."""
    import concourse.bass as bass
    import concourse.mybir as mybir
    import concourse.tile as tile
    from concourse import bacc

    dt = mybir.dt
    f32, bf16 = dt.float32, dt.bfloat16
    AF = mybir.ActivationFunctionType

    SB = S * BL          # tokens, b-major: sb = b*S + s
    KO = QK2 // P        # contraction subtiles
    HT = H // P          # h tiles
    CH = SB // CW        # token chunks (each chunk: one b, CW s-values)
    SBLK = S // P        # s blocks (final matmul contraction tiles)
    OCW = min(CW, DV)    # output free-dim chunk
    DT = DV // OCW
    TPC = CW // P        # transposed score sub-blocks per chunk
    KQ = max(1, KO // 4)  # xt DMA split granularity
    assert SB % CW == 0 and QK2 % P == 0 and H % P == 0 and S % P == 0
    assert S % CW == 0 and CW % P == 0

    nc = bacc.Bacc("TRN2", debug=False, target_bir_lowering=False)

    # chunk-major host layouts: each SBUF partition reads one contiguous
    # slab per DMA (minimal descriptor count, full queue bandwidth)
    xt = nc.dram_tensor("xt", [CH, P, KO, CW], bf16, kind="ExternalInput").ap()
    wct = nc.dram_tensor("wct", [P, KO, H], bf16, kind="ExternalInput").ap()
    # wv columns per h-tile, plus a trailing all-ones column
    wv = nc.dram_tensor("wv", [P, HT + 1], bf16, kind="ExternalInput").ap()
    vals = nc.dram_tensor("vals", [P, SBLK, DV], bf16, kind="ExternalInput").ap()
    out = nc.dram_tensor("out", [BL, DV], f32, kind="ExternalOutput").ap()

    with tile.TileContext(nc) as tc:
        with tc.tile_pool(name="const", bufs=1) as const_pool, \
             tc.tile_pool(name="xtp", bufs=XT_BUFS) as xt_pool, \
             tc.tile_pool(name="fp", bufs=2 * HT + 2) as f_pool, \
             tc.tile_pool(name="obs", bufs=2) as obs_pool, \
             tc.tile_pool(name="misc", bufs=2) as misc_pool, \
             tc.tile_pool(name="zps", bufs=Z_BUFS, space="PSUM") as zpsum, \
             tc.tile_pool(name="tps", bufs=2, space="PSUM") as tpsum, \
             tc.tile_pool(name="dps", bufs=1, space="PSUM") as dpsum, \
             tc.tile_pool(name="ops", bufs=2, space="PSUM") as opsum:

            # first xt chunk starts streaming before everything else so
            # the PE can begin the z stream as early as possible
            # separate tiles per ko slice / xt quarter so each z matmul
            # depends only on the one DMA that feeds it (whole-tile dep
            # tracking would otherwise gate the first matmul on ALL loads)
            NQ = (KO + KQ - 1) // KQ
            xt_tile0 = [xt_pool.tile([P, KQ, CW], bf16, tag=f"xtq{j}",
                                     name=f"xt0_q{j}") for j in range(NQ)]
            wct_ks = [const_pool.tile([P, H], bf16, name=f"wct_k{ko}")
                      for ko in range(KO)]
            wv_sb = const_pool.tile([P, HT + 1], bf16)
            for j in range(NQ):
                nc.sync.dma_start(xt_tile0[j][:],
                                  xt[0, :, j * KQ:(j + 1) * KQ, :])
                if j == 0:
                    nc.sync.dma_start(wct_ks[0][:], wct[:, 0, :])
                    nc.sync.dma_start(wv_sb[:], wv[:])
            for ko in range(1, KO):
                nc.sync.dma_start(wct_ks[ko][:], wct[:, ko, :])
            vals_sb = const_pool.tile([P, SBLK, DV], bf16)
            ones_col = wv_sb[:, HT:HT + 1]
            # exp(scores) in [s partitions, sblk, b] layout
            a_sb = const_pool.tile([P, SBLK, BL], bf16)

            # ACT-clock observer: a 1-element copy whose only dep is the
            # previous ACT output, so the following activation needs just
            # its PE wait.
            last_act = [None]

            def act_observe():
                if use_observers and last_act[0] is not None:
                    o = obs_pool.tile([1, 1], f32, tag="obs")
                    nc.scalar.activation(o[:], last_act[0], AF.Copy)

            SH = S // CW
            den_psum = dpsum.tile([BL, 1], f32)
            o_psums = [opsum.tile([BL, OCW], f32, tag="o", name=f"o_psum{d}") for d in range(DT)]

            def final_mms(sblk_range):
                for i, sblk in enumerate(sblk_range):
                    nc.tensor.matmul(
                        den_psum[:], lhsT=a_sb[:, sblk, :], rhs=ones_col,
                        start=(sblk == 0), stop=(sblk == SBLK - 1),
                        skip_group_check=True)
                for d in range(DT):
                    for sblk in sblk_range:
                        nc.tensor.matmul(
                            o_psums[d][:], lhsT=a_sb[:, sblk, :],
                            rhs=vals_sb[:, sblk, d * OCW:(d + 1) * OCW],
                            start=(sblk == 0), stop=(sblk == SBLK - 1),
                            skip_group_check=True)

            prev = None

            def emit_scores(f_tiles, shalf, b):
                for t in range(TPC):
                    tr_psum = tpsum.tile([P, 1], f32, tag="tr")
                    for h in range(HT):
                        nc.tensor.matmul(
                            tr_psum[:],
                            lhsT=f_tiles[h][:, t * P:(t + 1) * P],
                            rhs=wv_sb[:, h:h + 1],
                            start=(h == 0), stop=(h == HT - 1),
                            skip_group_check=True)
                    sblk = shalf * TPC + t
                    act_observe()
                    nc.scalar.activation(a_sb[:, sblk, b:b + 1], tr_psum[:],
                                         AF.Exp)
                    last_act[0] = a_sb[0:1, sblk, b:b + 1]

            for c in range(CH):
                shalf = c // BL
                b = c % BL
                if c == 0:
                    xt_tile = xt_tile0
                else:
                    cc = b * SH + shalf
                    xt_tile = [xt_pool.tile([P, KQ, CW], bf16, tag=f"xtq{j}",
                                            name=f"xt{c}_q{j}")
                               for j in range(NQ)]
                    for j in range(NQ):
                        nc.sync.dma_start(xt_tile[j][:],
                                          xt[cc, :, j * KQ:(j + 1) * KQ, :])
                if c == 2:
                    nc.sync.dma_start(vals_sb[:], vals[:])
                f_tiles = []
                for h in range(HT):
                    z_psum = zpsum.tile([P, CW], f32)
                    for ko in range(KO):
                        nc.tensor.matmul(
                            z_psum[:],
                            lhsT=wct_ks[ko][:, h * P:(h + 1) * P],
                            rhs=xt_tile[ko // KQ][:, ko % KQ, :],
                            start=(ko == 0), stop=(ko == KO - 1))
                    f_tile = f_pool.tile([P, CW], bf16)
                    act_observe()
                    nc.scalar.activation(f_tile[:], z_psum[:], AF.Tanh)
                    last_act[0] = f_tile[0:1, 0:1]
                    f_tiles.append(f_tile)
                # score phase pipelined one chunk behind the z stream so
                # the in-order PE never stalls on the current chunk's last
                # tanh (head-of-line blocking at chunk boundaries)
                if prev is not None:
                    emit_scores(*prev)
                prev = (f_tiles, shalf, b)
                if c > 0 and c % BL == 0:
                    # scores for all chunks of the previous s-half are now
                    # emitted: run that slice of the output matmuls
                    final_mms(range((shalf - 1) * TPC, shalf * TPC))

            emit_scores(*prev)
            final_mms(range((SH - 1) * TPC, SH * TPC))
            den_inv = misc_pool.tile([BL, 1], f32, tag="dinv")
            nc.vector.reciprocal(den_inv[:], den_psum[:])
            # let ACT observe the DVE tick so the scaled output copies
            # carry only their PE wait
            obs_d = obs_pool.tile([1, 1], f32, tag="obs")
            nc.scalar.activation(obs_d[:], den_inv[0:1, 0:1], AF.Copy)

            for d in range(DT):
                o_psum = o_psums[d]
                o_sb = misc_pool.tile([BL, OCW], f32, tag=f"o{d}")
                act_observe()
                nc.scalar.activation(o_sb[:], o_psum[:], AF.Copy,
                                     scale=den_inv[:])
                last_act[0] = o_sb[0:1, 0:1]
                nc.sync.dma_start(out[:, d * OCW:(d + 1) * OCW], o_sb[:])
    return nc


def prep_in_maps(queries, keys, values, W_q, W_k, w_v, n_cores=CORES):
    """Host-side shard + transpose (b-major) + bf16 cast."""
    bf = ml_dtypes.bfloat16
    queries = np.asarray(queries, dtype=np.float32)
    keys = np.asarray(keys, dtype=np.float32)
    S, B, QK = queries.shape
    BL = B // n_cores
    H = np.asarray(W_q).shape[0]
    HT = H // P

    q_bf = queries.astype(bf)
    k_bf = keys.astype(bf)
    KO = 2 * QK // P
    wct_np = np.ascontiguousarray(
        np.concatenate([np.asarray(W_q, np.float32),
                        np.asarray(W_k, np.float32)], axis=1).T
        .astype(bf).reshape(KO, P, H).transpose(1, 0, 2))
    wv_np = np.empty((P, HT + 1), dtype=bf)
    wv_np[:, :HT] = np.asarray(w_v, np.float32).reshape(HT, P).T.astype(bf)
    wv_np[:, HT] = np.float32(1.0)
    DV = np.asarray(values).shape[2]
    vals_np = np.ascontiguousarray(
        np.asarray(values, np.float32)[:, 0, :].astype(bf)
        .reshape(S // P, P, DV).transpose(1, 0, 2))

    in_maps = []
    for c in range(n_cores):
        # [S, BL, QK] -> [QK, BL, S] -> [QK, BL*S]   (sb = b*S + s)
        qT = np.ascontiguousarray(
            q_bf[:, c * BL:(c + 1) * BL, :].transpose(2, 1, 0)).reshape(QK, S * BL)
        kT = np.ascontiguousarray(
            k_bf[:, c * BL:(c + 1) * BL, :].transpose(2, 1, 0)).reshape(QK, S * BL)
        xt_2d = np.concatenate([qT, kT], axis=0)       # [2QK, BL*S]
        CW = CHUNK_W
        CH = S * BL // CW
        xt_np = np.ascontiguousarray(
            xt_2d.reshape(KO, P, CH, CW).transpose(2, 1, 0, 3))
        in_maps.append({"xt": xt_np, "wct": wct_np, "wv": wv_np,
                        "vals": vals_np})
    return in_maps


_NC_CACHE = {}


def _get_nc():
    if "nc" not in _NC_CACHE:
        nc = build_nc()
        nc.finalize()
        _NC_CACHE["nc"] = nc
    return _NC_CACHE["nc"]


def kernel_with_results(trace=False, **inputs):
    from concourse.bass_utils import run_bass_kernel_spmd
    nc = _get_nc()
    in_maps = prep_in_maps(**inputs)
    res = run_bass_kernel_spmd(nc, in_maps, core_ids=list(range(CORES)),
                               trace=trace)
    out = np.concatenate([np.asarray(res.results[i]["out"], np.float32)
                          for i in range(CORES)], axis=0)
    return out, res


def kernel(**inputs):
    out, _ = kernel_with_results(trace=False, **inputs)
    return out



# revision 2
# speedup vs baseline: 1.6459x; 1.6459x over previous
"""Additive attention (Bahdanau-style) on 8 TRN2 NeuronCores — fp8 DoubleRow.

Reference computation (S=1024, B=64, QK=H=DV=1024):
    q = queries @ W_q.T          [S,B,H]
    k = keys    @ W_k.T          [S,B,H]
    f = tanh(q + k)              [S,B,H]
    scores = f @ w_v.T           [S,B,1]
    attn = softmax(scores, axis=S)
    out[b,d] = sum_s attn[s,b] * values[s,0,d]    -> [B,DV]

Strategy: pure data parallel over B (8 batches per core), weights/values
replicated, no collectives.  The dominant work is the fused projection
z = [W_q W_k] @ [q;k] ([2QK=2048] contraction, H=1024 outputs, SB=8192
tokens/core).  v2 computes 7 of the 8 h-tiles with fp8-e4m3 matmuls in
perf_mode=DoubleRow (2 contraction rows per PE cell -> measured ~569
cycles per 256-deep N=512 matmul ~= 1.92x bf16 throughput; LDWEIGHTS
hides behind the stream).  Accuracy is recovered by:
  * host-side permutation of the H channels by |w_v[h]|: the top 128
    channels (which carry ~half the score-error energy) go to the last
    h-tile, computed in bf16 from a separate bf16 x-stream;
  * a rank-1 "mean-field" correction of the score logits:
        score_err ~= sum_h w_v[h]*sech^2(z_h)*dz_h,  sech^2 ~= c=0.47
        corr[s,b] = c*(u.x[s,b] + v.dx[s,b]),  u = w_v^T dW, v = w_v^T W8
    computed host-side (two GEMVs) and added as the per-partition bias
    of the on-device Exp activation.
Measured rel err vs the fp32 reference: ~1.4e-2 (gate 2e-2); bf16
baseline was 3.5e-3 at 487 us.

Layouts (host-prepped, chunk-major so each SBUF partition reads one
contiguous slab per DMA): xt8/xtb [CH,P,KO,CW] fp8/bf16 with contraction
index k = ko*128+p and token sb = b*S+s; wct8 [P,KO,H7] fp8 (tiles 0-6),
wctb [P,KO,128] bf16 (tile 7); DoubleRow matmuls consume ko-pair slices
[:, 2j:2j+2, :] (3D AP, middle dim = the 2 per-cell weights).  Scores are
produced transposed ([s,b] layout) by f-stationary N=1 matmuls as in the
bf16 baseline; softmax normalization folds into the output copy scale.

The walrus build rejects engine instructions with more than one embedded
sync wait, so the program is built as bacc.Bacc and run through
Bacc.compile() (generate_event_semaphores splits excess waits).
"""

import numpy as np
import ml_dtypes

P = 128
CORES = 8
CHUNK_W = 512   # token-chunk width; build_nc and prep_in_maps must agree
NTOP = 128      # |w_v|-top channels computed in bf16 (must be 1 h-tile)
CORR_C = 0.47   # mean-field sech^2 coefficient of the logit correction

FULL_S, FULL_B, FULL_QK, FULL_H, FULL_DV = 1024, 64, 1024, 1024, 1024


def build_nc(S=FULL_S, BL=FULL_B // CORES, QK2=2 * FULL_QK, H=FULL_H,
             DV=FULL_DV, CW=CHUNK_W, XT_BUFS=4, Z_BUFS=3):
    """Build the single-core Bacc program (same NEFF runs SPMD on all cores)."""
    import concourse.bass as bass
    import concourse.mybir as mybir
    import concourse.tile as tile
    from concourse import bacc

    dt = mybir.dt
    f32, bf16, f8 = dt.float32, dt.bfloat16, dt.float8e4
    AF = mybir.ActivationFunctionType
    DR = mybir.MatmulPerfMode.DoubleRow

    SB = S * BL          # tokens, b-major: sb = b*S + s
    KO = QK2 // P        # contraction subtiles (16)
    JO = KO // 2         # DoubleRow pair count (8)
    HT = H // P          # h tiles (8)
    HF = HT - 1          # fp8 h-tiles (7)
    H7 = HF * P          # fp8 channels (896)
    CH = SB // CW        # token chunks (each chunk: one b, CW s-values)
    SBLK = S // P        # s blocks (final matmul contraction tiles)
    OCW = min(CW, DV)    # output free-dim chunk
    DT = DV // OCW
    TPC = CW // P        # transposed score sub-blocks per chunk
    KQ = max(1, KO // 4)  # xt DMA split granularity (4 ko per quarter)
    NQ = (KO + KQ - 1) // KQ
    assert SB % CW == 0 and QK2 % P == 0 and H % P == 0 and S % P == 0
    assert S % CW == 0 and CW % P == 0 and KQ % 2 == 0

    nc = bacc.Bacc("TRN2", debug=False, target_bir_lowering=False)

    xt8 = nc.dram_tensor("xt8", [CH, P, KO, CW], f8, kind="ExternalInput").ap()
    xtb = nc.dram_tensor("xtb", [CH, P, KO, CW], bf16, kind="ExternalInput").ap()
    wct8 = nc.dram_tensor("wct8", [P, KO, H7], f8, kind="ExternalInput").ap()
    wctb = nc.dram_tensor("wctb", [P, KO, P], bf16, kind="ExternalInput").ap()
    # wv columns per h-tile (permuted channel order), plus an all-ones column
    wv = nc.dram_tensor("wv", [P, HT + 1], bf16, kind="ExternalInput").ap()
    vals = nc.dram_tensor("vals", [P, SBLK, DV], bf16, kind="ExternalInput").ap()
    corr = nc.dram_tensor("corr", [P, SBLK, BL], f32, kind="ExternalInput").ap()
    out = nc.dram_tensor("out", [BL, DV], f32, kind="ExternalOutput").ap()

    with tile.TileContext(nc) as tc:
        with tc.tile_pool(name="const", bufs=1) as const_pool, \
             tc.tile_pool(name="xtp8", bufs=XT_BUFS) as xt8_pool, \
             tc.tile_pool(name="xtpb", bufs=XT_BUFS) as xtb_pool, \
             tc.tile_pool(name="fp", bufs=2 * HT + 2) as f_pool, \
             tc.tile_pool(name="misc", bufs=2) as misc_pool, \
             tc.tile_pool(name="zps", bufs=Z_BUFS, space="PSUM") as zpsum, \
             tc.tile_pool(name="tps", bufs=2, space="PSUM") as tpsum, \
             tc.tile_pool(name="dps", bufs=1, space="PSUM") as dpsum, \
             tc.tile_pool(name="ops", bufs=2, space="PSUM") as opsum:

            # first xt chunk starts streaming before everything else so
            # the PE can begin the z stream as early as possible.
            # separate tiles per xt quarter so each z matmul depends only
            # on the one DMA that feeds it.
            xt8_t0 = [xt8_pool.tile([P, KQ, CW], f8, tag=f"x8q{j}",
                                    name=f"x8_0q{j}") for j in range(NQ)]
            xtb_t0 = [xtb_pool.tile([P, KQ, CW], bf16, tag=f"xbq{j}",
                                    name=f"xb_0q{j}") for j in range(NQ)]
            w8_js = [const_pool.tile([P, 2, H7], f8, name=f"w8_j{j}")
                     for j in range(JO)]
            wb_sb = const_pool.tile([P, KO, P], bf16)
            wv_sb = const_pool.tile([P, HT + 1], bf16)
            corr_sb = const_pool.tile([P, SBLK, BL], f32)
            for j in range(NQ):
                nc.sync.dma_start(xt8_t0[j][:], xt8[0, :, j * KQ:(j + 1) * KQ, :])
                if j == 0:
                    nc.sync.dma_start(w8_js[0][:], wct8[:, 0:2, :])
                    nc.sync.dma_start(wv_sb[:], wv[:])
                    nc.sync.dma_start(corr_sb[:], corr[:])
            for j in range(1, JO):
                nc.sync.dma_start(w8_js[j][:], wct8[:, 2 * j:2 * j + 2, :])
            nc.sync.dma_start(wb_sb[:], wctb[:])
            for j in range(NQ):
                nc.sync.dma_start(xtb_t0[j][:], xtb[0, :, j * KQ:(j + 1) * KQ, :])
            vals_sb = const_pool.tile([P, SBLK, DV], bf16)
            ones_col = wv_sb[:, HT:HT + 1]
            # exp(scores+corr) in [s partitions, sblk, b] layout
            a_sb = const_pool.tile([P, SBLK, BL], bf16)

            SH = S // CW
            den_psum = dpsum.tile([BL, 1], f32)
            o_psums = [opsum.tile([BL, OCW], f32, tag="o", name=f"o_psum{d}")
                       for d in range(DT)]

            def final_mms(sblk_range):
                for sblk in sblk_range:
                    nc.tensor.matmul(
                        den_psum[:], lhsT=a_sb[:, sblk, :], rhs=ones_col,
                        start=(sblk == 0), stop=(sblk == SBLK - 1),
                        skip_group_check=True)
                for d in range(DT):
                    for sblk in sblk_range:
                        nc.tensor.matmul(
                            o_psums[d][:], lhsT=a_sb[:, sblk, :],
                            rhs=vals_sb[:, sblk, d * OCW:(d + 1) * OCW],
                            start=(sblk == 0), stop=(sblk == SBLK - 1),
                            skip_group_check=True)

            prev = None

            def emit_scores(f_tiles, shalf, b):
                for t in range(TPC):
                    tr_psum = tpsum.tile([P, 1], f32, tag="tr")
                    for h in range(HT):
                        nc.tensor.matmul(
                            tr_psum[:],
                            lhsT=f_tiles[h][:, t * P:(t + 1) * P],
                            rhs=wv_sb[:, h:h + 1],
                            start=(h == 0), stop=(h == HT - 1),
                            skip_group_check=True)
                    sblk = shalf * TPC + t
                    nc.scalar.activation(a_sb[:, sblk, b:b + 1], tr_psum[:],
                                         AF.Exp, bias=corr_sb[:, sblk, b:b + 1])

            for c in range(CH):
                shalf = c // BL
                b = c % BL
                if c == 0:
                    xt8_t, xtb_t = xt8_t0, xtb_t0
                else:
                    cc = b * SH + shalf
                    xt8_t = [xt8_pool.tile([P, KQ, CW], f8, tag=f"x8q{j}",
                                           name=f"x8_{c}q{j}")
                             for j in range(NQ)]
                    xtb_t = [xtb_pool.tile([P, KQ, CW], bf16, tag=f"xbq{j}",
                                           name=f"xb_{c}q{j}")
                             for j in range(NQ)]
                    for j in range(NQ):
                        nc.sync.dma_start(xt8_t[j][:],
                                          xt8[cc, :, j * KQ:(j + 1) * KQ, :])
                        nc.sync.dma_start(xtb_t[j][:],
                                          xtb[cc, :, j * KQ:(j + 1) * KQ, :])
                if c == 2:
                    nc.sync.dma_start(vals_sb[:], vals[:])
                f_tiles = []
                for h in range(HF):          # fp8 DoubleRow tiles 0..6
                    z_psum = zpsum.tile([P, CW], f32)
                    for j in range(JO):
                        q, r = divmod(2 * j, KQ)
                        nc.tensor.matmul(
                            z_psum[:],
                            lhsT=w8_js[j][:, :, h * P:(h + 1) * P],
                            rhs=xt8_t[q][:, r:r + 2, :],
                            perf_mode=DR,
                            start=(j == 0), stop=(j == JO - 1))
                    f_tile = f_pool.tile([P, CW], bf16)
                    nc.scalar.activation(f_tile[:], z_psum[:], AF.Tanh)
                    f_tiles.append(f_tile)
                # bf16 top-|w_v| tile (channel-permuted to tile HT-1)
                z_psum = zpsum.tile([P, CW], f32)
                for ko in range(KO):
                    q, r = divmod(ko, KQ)
                    nc.tensor.matmul(
                        z_psum[:], lhsT=wb_sb[:, ko, :],
                        rhs=xtb_t[q][:, r, :],
                        start=(ko == 0), stop=(ko == KO - 1))
                f_tile = f_pool.tile([P, CW], bf16)
                nc.scalar.activation(f_tile[:], z_psum[:], AF.Tanh)
                f_tiles.append(f_tile)
                # score phase pipelined one chunk behind the z stream so
                # the in-order PE never stalls on the current chunk's last
                # tanh (head-of-line blocking at chunk boundaries)
                if prev is not None:
                    emit_scores(*prev)
                prev = (f_tiles, shalf, b)
                if c > 0 and c % BL == 0:
                    # scores for all chunks of the previous s-half are now
                    # emitted: run that slice of the output matmuls
                    final_mms(range((shalf - 1) * TPC, shalf * TPC))

            emit_scores(*prev)
            final_mms(range((SH - 1) * TPC, SH * TPC))
            den_inv = misc_pool.tile([BL, 1], f32, tag="dinv")
            nc.vector.reciprocal(den_inv[:], den_psum[:])

            for d in range(DT):
                o_psum = o_psums[d]
                o_sb = misc_pool.tile([BL, OCW], f32, tag=f"o{d}")
                nc.scalar.activation(o_sb[:], o_psum[:], AF.Copy,
                                     scale=den_inv[:])
                nc.sync.dma_start(out[:, d * OCW:(d + 1) * OCW], o_sb[:])
    return nc


def prep_in_maps(queries, keys, values, W_q, W_k, w_v, n_cores=CORES):
    """Host-side shard + permute + transpose (b-major) + fp8/bf16 cast +
    rank-1 logit correction."""
    bf = ml_dtypes.bfloat16
    f8 = ml_dtypes.float8_e4m3
    queries = np.asarray(queries, dtype=np.float32)
    keys = np.asarray(keys, dtype=np.float32)
    S, B, QK = queries.shape
    BL = B // n_cores
    W_q = np.asarray(W_q, np.float32)
    W_k = np.asarray(W_k, np.float32)
    w_v = np.asarray(w_v, np.float32)
    H = W_q.shape[0]
    HT = H // P
    H7 = H - NTOP
    KO = 2 * QK // P
    CW = CHUNK_W
    SBLK = S // P

    # channel permutation: fp8 channels first, top-|w_v| last (tile HT-1)
    order = np.argsort(-np.abs(w_v[0]))
    perm = np.concatenate([order[NTOP:], order[:NTOP]])
    Wc = np.concatenate([W_q, W_k], axis=1)[perm]      # [H, 2QK]
    wv_p = w_v[0, perm]                                # [H]

    W8 = Wc[:H7].astype(f8)
    W8f = W8.astype(np.float32)
    # wct8 [P, KO, H7]: contraction k = ko*P + p
    wct8_np = np.ascontiguousarray(
        W8.T.reshape(KO, P, H7).transpose(1, 0, 2))
    wctb_np = np.ascontiguousarray(
        Wc[H7:].astype(bf).T.reshape(KO, P, NTOP).transpose(1, 0, 2))

    wv_np = np.empty((P, HT + 1), dtype=bf)
    wv_np[:, :HT] = wv_p.reshape(HT, P).T.astype(bf)
    wv_np[:, HT] = np.float32(1.0)

    DV = np.asarray(values).shape[2]
    vals_np = np.ascontiguousarray(
        np.asarray(values, np.float32)[:, 0, :].astype(bf)
        .reshape(S // P, P, DV).transpose(1, 0, 2))

    # rank-1 correction vectors (fp8 channels only; bf16 tile's x-error
    # is at the bf16 noise floor)
    dW = Wc[:H7] - W8f
    u = (CORR_C * (wv_p[:H7] @ dW)).astype(np.float32)      # [2QK]
    v = (CORR_C * (wv_p[:H7] @ W8f)).astype(np.float32)     # [2QK]

    in_maps = []
    CHb = S * BL // CW
    for c in range(n_cores):
        # [S, BL, QK] -> [QK, BL, S] -> [QK, BL*S]   (sb = b*S + s)
        qT = np.ascontiguousarray(
            queries[:, c * BL:(c + 1) * BL, :].transpose(2, 1, 0)
        ).reshape(QK, S * BL)
        kT = np.ascontiguousarray(
            keys[:, c * BL:(c + 1) * BL, :].transpose(2, 1, 0)
        ).reshape(QK, S * BL)
        xt_2d = np.concatenate([qT, kT], axis=0)       # [2QK, BL*S] f32
        x8_2d = xt_2d.astype(f8)
        xt8_np = np.ascontiguousarray(
            x8_2d.reshape(KO, P, CHb, CW).transpose(2, 1, 0, 3))
        xtb_np = np.ascontiguousarray(
            xt_2d.astype(bf).reshape(KO, P, CHb, CW).transpose(2, 1, 0, 3))
        # corr[s, b] = u.x + v.dx  (token-major: columns of xt_2d)
        dx_2d = xt_2d - x8_2d.astype(np.float32)
        corr_tok = u @ xt_2d + v @ dx_2d               # [BL*S]
        corr_np = np.ascontiguousarray(
            corr_tok.reshape(BL, SBLK, P).transpose(2, 1, 0)
        ).astype(np.float32)
        in_maps.append({"xt8": xt8_np, "xtb": xtb_np, "wct8": wct8_np,
                        "wctb": wctb_np, "wv": wv_np, "vals": vals_np,
                        "corr": corr_np})
    return in_maps


_NC_CACHE = {}


def _get_nc():
    if "nc" not in _NC_CACHE:
        nc = build_nc()
        nc.finalize()
        _NC_CACHE["nc"] = nc
    return _NC_CACHE["nc"]


def kernel_with_results(trace=False, **inputs):
    from concourse.bass_utils import run_bass_kernel_spmd
    nc = _get_nc()
    in_maps = prep_in_maps(**inputs)
    res = run_bass_kernel_spmd(nc, in_maps, core_ids=list(range(CORES)),
                               trace=trace)
    out = np.concatenate([np.asarray(res.results[i]["out"], np.float32)
                          for i in range(CORES)], axis=0)
    return out, res


def kernel(**inputs):
    out, _ = kernel_with_results(trace=False, **inputs)
    return out
